# revision 1
# baseline (speedup 1.0000x reference)
"""CNLinkPredictor Trainium2 kernel.

Edge-sharded across 8 NeuronCores (1024 target edges each); x, adj, and the
MLP weights are replicated. Per core:
  A) h = x + MLP(x) computed in transposed layout: the host supplies xT, so
     stage A is matmul-only on PE (bf16, N=512 moving), fused bias+ReLU on
     the scalar engine, residual on DVE, then xbar DMA-transposes write h
     back to natural layout (bf16, (half, ktile, c) column order so every
     transpose destination is a contiguous per-partition span).
  B) per 128-edge block and k-half: indirect-DMA gather of the two adjacency
     rows per edge (fp8 - exact for a 0/1 adjacency - one row per SBUF
     partition), DVE multiply -> cn (bf16, still exact), one xbar
     DMA-transpose, then 32 matmuls accumulating cnT @ h into PSUM.
  C) edge MLPs in transposed layout (bf16, N=512 over 4-block groups), xbar
     transposes for xcn and xi*xj, final [1, 1024] output row.

Emission is software-pipelined (A first half, B k-half 0, A second half,
B k-half 1, C, ...) so the FIFO engine queues never head-of-line block on
data that is not ready yet.

Hardware pitfalls this kernel works around:
  - This walrus build accepts at most ONE sync-wait per instruction
    (_apply_tile_patch splits the Tile tail drain; _split_multi_waits hoists
    extra waits onto same-engine NoOps).
  - Concurrent 4-byte DMA traffic corrupts in-flight 2-byte xbar
    DMA-transposes, so every steady-state transfer is <= 2 bytes/element
    (fp8 adjacency, bf16 everything else); the few f32/int32 loads happen
    up front and the single f32 store happens after the last transpose.
  - xbar transposes into non-contiguous destinations produce wrong data;
    all transpose targets collapse to contiguous 2-D access patterns.
"""

import numpy as np
import ml_dtypes

N = 8192
C = 256
E = 8192
NCORES = 8
EL = E // NCORES          # edges per core
P = 128
NB = EL // P              # edge blocks per core
KH = 2                    # k halves for adjacency gather
KC = N // KH              # columns per half
NKT = N // P              # 64 k tiles
AGRP = 512                # stage-A node group
CGRP = 4                  # stage-C blocks per group (512 edges)

_CACHE = {}
TRACE = False
LAST_RESULT = None
DEBUG_DUMPS = False


def _apply_tile_patch():
    """Split the Tile tail-drain's multi-sem wait onto individual SP nops."""
    from concourse.tile import TileContext
    from concourse.vector_clock import ScopedClock

    if getattr(TileContext, "_drain_patched", False):
        return

    def _patched(self, tick_clock, wait_clock):
        nc = self.nc
        collector = nc.sync.nop()
        wait_clock.add_sem_waits(
            collector.ins, ScopedClock({None: tick_clock.global_clock})
        )
        si = collector.ins.sync_info
        waits = list(si.on_wait) if si is not None and si.on_wait else []
        if si is not None and len(waits) > 1:
            name_to_handle = {h.name: h for h in self.sems.allocated().values()}
            si.on_wait = [waits[0]]
            for w in waits[1:]:
                op = {
                    "sem-ge-imm": "sem-ge",
                    "sem-eq-imm": "sem-eq",
                    "sem-le-imm": "sem-le",
                }.get(str(w.wait_mode), "sem-ge")
                nc.sync.nop().wait_op(name_to_handle[w.ant_name], w.wait_value, op)
        nc.sync.drain()
        nc.all_engine_barrier()
        assert self.sems is not None
        popped = nc._tile_sem_poison_stack.pop()
        assert popped is self._sem_poison
        nc.clear_and_free_semaphores(list(self.sems.allocated().values()))
        nc.all_engine_barrier()

    TileContext._drain_and_barrier = _patched
    TileContext._drain_patched = True


def _split_multi_waits(nc):
    """Hoist extra sync-waits onto same-engine NoOps (sequential waits ==
    ANDed waits); this walrus build allows one wait per instruction."""
    import concourse.mybir as mybir

    cnt = 0
    for fn in nc.m.functions:
        for bb in fn.blocks:
            out = []
            for inst in bb.instructions:
                si = getattr(inst, "sync_info", None)
                waits = list(si.on_wait) if si is not None and si.on_wait else []
                if len(waits) > 1:
                    for w in waits[:-1]:
                        nop = mybir.InstNoOp(name=f"ws-{cnt}", ins=[], outs=[])
                        cnt += 1
                        nop.engine = inst.engine
                        nop.sync_info = mybir.SyncInfo(on_wait=[w], on_update=[])
                        out.append(nop)
                    si.on_wait = [waits[-1]]
                out.append(inst)
            bb.instructions = out
    return nc


def _build(split_waits=True):
    import concourse.bass as bass
    import concourse.mybir as mybir
    from concourse.tile import TileContext

    _apply_tile_patch()

    f32 = mybir.dt.float32
    f32r = mybir.dt.float32r
    bf16 = mybir.dt.bfloat16
    fp8 = mybir.dt.float8e4
    i32 = mybir.dt.int32
    Relu = mybir.ActivationFunctionType.Relu
    Ident = mybir.ActivationFunctionType.Identity
    MUL = mybir.AluOpType.mult
    ADD = mybir.AluOpType.add

    nc = bass.Bass(num_swdge_queues=4)

    xT_d = nc.dram_tensor("xT", [C, N], bf16, kind="ExternalInput")
    x_d = nc.dram_tensor("x", [N, C], bf16, kind="ExternalInput")
    adj_d = nc.dram_tensor("adj", [N, N], fp8, kind="ExternalInput")
    idx_d = nc.dram_tensor("idx", [2, EL], i32, kind="ExternalInput")
    # all matmul weights in bf16 (2-byte rule; see module docstring)
    wA = {n: nc.dram_tensor(n, [C, C], bf16, kind="ExternalInput")
          for n in ("xlin_w1", "xlin_w2")}
    wC = {n: nc.dram_tensor(n, [C, C], bf16, kind="ExternalInput")
          for n in ("xcn_w1", "xcn_w2", "xij_w", "lin_w1")}
    lin_w2_d = nc.dram_tensor("lin_w2", [C, 1], bf16, kind="ExternalInput")
    bnames = ["xlin_b1", "xlin_b2", "xcn_b1", "xcn_b2", "xij_b", "lin_b1"]
    ball_d = nc.dram_tensor("ball", [P, 2 * len(bnames)], f32,
                            kind="ExternalInput")
    lin_b2_d = nc.dram_tensor("lin_b2", [1, 1], f32, kind="ExternalInput")
    beta_d = nc.dram_tensor("beta_bc", [P, 1], f32, kind="ExternalInput")
    out_d = nc.dram_tensor("out", [1, EL], f32, kind="ExternalOutput")
    dbg = {}
    if DEBUG_DUMPS:
        dbg["h_all"] = nc.dram_tensor("dbg_h", [P, 2 * N], bf16,
                                      kind="ExternalOutput")
        dbg["cn"] = nc.dram_tensor("dbg_cn", [P, KC], bf16,
                                   kind="ExternalOutput")
        dbg["cnT"] = nc.dram_tensor("dbg_cnT", [P, KC], bf16,
                                    kind="ExternalOutput")
        dbg["xcn"] = nc.dram_tensor("dbg_xcn", [P, C], bf16,
                                    kind="ExternalOutput")
        dbg["xcnT"] = nc.dram_tensor("dbg_xcnT", [P, 2 * CGRP * P], bf16,
                                     kind="ExternalOutput")
        dbg["prodT"] = nc.dram_tensor("dbg_prodT", [P, 2 * CGRP * P], bf16,
                                      kind="ExternalOutput")

    _swq = [0]

    def _rr(inst):
        q = _swq[0] % 4
        _swq[0] += 1
        if q:
            inst.ins.queue = f"qPoolDynamic{q}"
        return inst

    with TileContext(nc) as tc:
        with (
            tc.tile_pool(name="const", bufs=1) as pK,
            tc.tile_pool(name="hpool", bufs=1) as pH,
            tc.tile_pool(name="adj", bufs=5) as pAdj,
            tc.tile_pool(name="cn", bufs=4) as pCn,
            tc.tile_pool(name="cnT", bufs=4) as pT,
            tc.tile_pool(name="edge", bufs=2) as pC,
            tc.tile_pool(name="xcn", bufs=CGRP) as pX,
        ):
            # ---- constants ----
            # idx first: the stage-B gathers depend only on these
            idx_sb = pK.tile([P, 2 * NB], i32, tag="idx_sb", name="idx_sb")
            nc.sync.dma_start(
                out=idx_sb[:].rearrange("p (t b) -> p t b", t=2),
                in_=idx_d[:, :].rearrange("t (b p) -> p t b", p=P),
            )
            ii = [idx_sb[:, b:b + 1] for b in range(NB)]
            jj = [idx_sb[:, NB + b:NB + b + 1] for b in range(NB)]

            wA_sb, wC_sb = {}, {}
            for n, t_d in list(wA.items()) + list(wC.items()):
                t = pK.tile([P, 2 * C], bf16, tag=f"w_{n}", name=f"w_{n}")
                nc.sync.dma_start(
                    out=t[:].rearrange("p (k n2) -> p k n2", k=2),
                    in_=t_d[:, :].rearrange("(k p) n2 -> p k n2", p=P),
                )
                pair = [t[:, 0:C], t[:, C:2 * C]]
                (wA_sb if n in wA else wC_sb)[n] = pair
            lw2_t = pK.tile([P, 2], bf16, tag="lin_w2", name="lin_w2t")
            nc.sync.dma_start(
                out=lw2_t[:].rearrange("p (k o) -> p k o", k=2),
                in_=lin_w2_d[:, :].rearrange("(k p) o -> p k o", p=P),
            )
            lw2_sb = [lw2_t[:, 0:1], lw2_t[:, 1:2]]
            b_sb = {}
            ball = pK.tile([P, 2 * len(bnames)], f32, tag="ball", name="ball")
            nc.sync.dma_start(
                out=ball[:],
                in_=ball_d[:, :],
            )
            for q, n in enumerate(bnames):
                b_sb[n] = ball[:, 2 * q:2 * q + 2]
            lb2_sb = pK.tile([1, 1], f32, tag="b_lin2", name="b_lin2")
            nc.sync.dma_start(out=lb2_sb[:], in_=lin_b2_d[:, :])
            beta_sb = pK.tile([P, 1], f32, tag="beta", name="beta")
            nc.sync.dma_start(out=beta_sb[:], in_=beta_d[:, :])

            out_row = pK.tile([1, EL], f32, tag="out_row", name="out_row")
            # natural-layout h in (hh, kt, c2) order so the xbar transposes
            # write contiguous per-partition spans: column = hh*N + kt*128 + c2
            # encodes h[node = kt*128 + p, channel = hh*128 + c2].
            h_all = pH.tile([P, 2 * N], bf16, tag="h_all", name="h_all")
            h_view = h_all[:].rearrange("p (hh kt c) -> p hh kt c", hh=2, c=P)

            # ---- stage definitions ----
            def stage_a_group(g, pA, psA):
                m0 = g * AGRP
                xT = []
                for h in range(2):
                    t = pA.tile([P, AGRP], bf16, tag=f"xT{h}", name=f"xT{h}_{g}")
                    nc.scalar.dma_start(
                        out=t[:], in_=xT_d[h * P:(h + 1) * P, m0:m0 + AGRP]
                    )
                    xT.append(t)
                y1T = []
                for h in range(2):
                    ps = psA.tile([P, AGRP], f32, tag="psmm", name=f"psA1_{g}{h}")
                    nc.tensor.matmul(
                        ps[:], wA_sb["xlin_w1"][0][:, h * P:(h + 1) * P],
                        xT[0][:], start=True, stop=False,
                    )
                    nc.tensor.matmul(
                        ps[:], wA_sb["xlin_w1"][1][:, h * P:(h + 1) * P],
                        xT[1][:], start=False, stop=True,
                    )
                    t = pA.tile([P, AGRP], bf16, tag=f"y1T{h}", name=f"y1T{h}_{g}")
                    nc.scalar.activation(
                        t[:], ps[:], Relu, bias=b_sb["xlin_b1"][:, h:h + 1]
                    )
                    y1T.append(t)
                for h in range(2):
                    ps = psA.tile([P, AGRP], f32, tag="psmm", name=f"psA2_{g}{h}")
                    nc.tensor.matmul(
                        ps[:], wA_sb["xlin_w2"][0][:, h * P:(h + 1) * P],
                        y1T[0][:], start=True, stop=False,
                    )
                    nc.tensor.matmul(
                        ps[:], wA_sb["xlin_w2"][1][:, h * P:(h + 1) * P],
                        y1T[1][:], start=False, stop=True,
                    )
                    y2 = pA.tile([P, AGRP], bf16, tag="y2T", name=f"y2T{h}_{g}")
                    nc.scalar.activation(
                        y2[:], ps[:], Relu, bias=b_sb["xlin_b2"][:, h:h + 1]
                    )
                    hT = pA.tile([P, AGRP], bf16, tag=f"hT{h}", name=f"hT{h}_{g}")
                    nc.vector.tensor_tensor(
                        out=hT[:], in0=xT[h][:], in1=y2[:], op=ADD
                    )
                    nc.sync.dma_start_transpose(
                        out=h_view[:, h,
                                   g * (AGRP // P):(g + 1) * (AGRP // P), :],
                        in_=hT[:],
                    )

            xcn_sb = [None] * NB

            cnT_map = {}

            def stage_b_load(b, s):
                ai = pAdj.tile([P, KC], fp8, tag="ai", name=f"ai{b}_{s}")
                _rr(nc.gpsimd.indirect_dma_start(
                    out=ai[:], out_offset=None, in_=adj_d[:, :],
                    in_offset=bass.IndirectOffsetOnAxis(ap=ii[b][:, :1], axis=0),
                    element_offset=s * KC,
                ))
                aj = pAdj.tile([P, KC], fp8, tag="aj", name=f"aj{b}_{s}")
                _rr(nc.gpsimd.indirect_dma_start(
                    out=aj[:], out_offset=None, in_=adj_d[:, :],
                    in_offset=bass.IndirectOffsetOnAxis(ap=jj[b][:, :1], axis=0),
                    element_offset=s * KC,
                ))
                cn = pCn.tile([P, KC], bf16, tag="cn", name=f"cn{b}_{s}")
                nc.vector.tensor_tensor(out=cn[:], in0=ai[:], in1=aj[:], op=MUL)
                cnT = pT.tile([P, KC], bf16, tag="cnT", name=f"cnT{b}_{s}")
                nc.sync.dma_start_transpose(
                    out=cnT[:].rearrange("p (kt e) -> p kt e", e=P),
                    in_=cn[:],
                )
                cnT_map[(b, s)] = cnT

            def stage_b_mms(b, s, psxcn):
                cnT = cnT_map[(b, s)]
                for kt in range(KC // P):
                    ktg = s * (KC // P) + kt
                    nc.tensor.matmul(
                        psxcn[:],
                        cnT[:, kt * P:(kt + 1) * P],
                        h_view[:, :, ktg, :],
                        start=(ktg == 0), stop=(ktg == NKT - 1),
                    )

            def stage_b_finish(b, psxcn):
                xcn_sb[b] = pX.tile([P, C], bf16, tag="xcn", name=f"xcn{b}")
                nc.vector.tensor_copy(xcn_sb[b][:], psxcn[:])

            prodT_map = {}

            def stage_c_prod(grp):
                blocks = range(grp * CGRP, (grp + 1) * CGRP)
                W = CGRP * P
                prodT = pC.tile([P, 2 * W], bf16, tag="prodT", name=f"prodT{grp}")
                prodT_v = prodT[:].rearrange(
                    "p (blk hh e) -> p blk hh e", blk=CGRP, e=P)
                prodT_map[grp] = prodT
                for t2, b in enumerate(blocks):
                    xi = pC.tile([P, C], bf16, tag="xi", name=f"xi{b}")
                    _rr(nc.gpsimd.indirect_dma_start(
                        out=xi[:], out_offset=None, in_=x_d[:, :],
                        in_offset=bass.IndirectOffsetOnAxis(
                            ap=ii[b][:, :1], axis=0),
                    ))
                    xj = pC.tile([P, C], bf16, tag="xj", name=f"xj{b}")
                    _rr(nc.gpsimd.indirect_dma_start(
                        out=xj[:], out_offset=None, in_=x_d[:, :],
                        in_offset=bass.IndirectOffsetOnAxis(
                            ap=jj[b][:, :1], axis=0),
                    ))
                    pt = pC.tile([P, C], bf16, tag="prod", name=f"prod{b}")
                    nc.vector.tensor_tensor(
                        out=pt[:], in0=xi[:], in1=xj[:], op=MUL
                    )
                    nc.sync.dma_start_transpose(
                        out=prodT_v[:, t2, :, :], in_=pt[:],
                    )

            def stage_c(grp, psC, psO):
                blocks = range(grp * CGRP, (grp + 1) * CGRP)
                W = CGRP * P  # 512 edges
                xcnT = pC.tile([P, 2 * W], bf16, tag="xcnT", name=f"xcnT{grp}")
                xcnT_v = xcnT[:].rearrange(
                    "p (blk hh e) -> p blk hh e", blk=CGRP, e=P)
                prodT = prodT_map[grp]
                for t2, b in enumerate(blocks):
                    nc.sync.dma_start_transpose(
                        out=xcnT_v[:, t2, :, :], in_=xcn_sb[b][:],
                    )

                def mlp_layer(rhs2, wname, bname, outtag, packed):
                    outs = []
                    for h in range(2):
                        ps = psC.tile([P, W], f32, tag="psc",
                                      name=f"psc_{grp}_{outtag}{h}")
                        if packed:
                            rhs_v = rhs2[:].rearrange(
                                "p (blk hh e) -> p blk hh e", blk=CGRP, e=P)
                            r0, r1 = rhs_v[:, :, 0, :], rhs_v[:, :, 1, :]
                        else:
                            r0, r1 = rhs2[0][:], rhs2[1][:]
                        nc.tensor.matmul(
                            ps[:], wC_sb[wname][0][:, h * P:(h + 1) * P],
                            r0, start=True, stop=False,
                        )
                        nc.tensor.matmul(
                            ps[:], wC_sb[wname][1][:, h * P:(h + 1) * P],
                            r1, start=False, stop=True,
                        )
                        t = pC.tile([P, W], bf16, tag=f"{outtag}{h}",
                                    name=f"{outtag}{h}_{grp}")
                        nc.scalar.activation(
                            t[:], ps[:], Relu, bias=b_sb[bname][:, h:h + 1]
                        )
                        outs.append(t)
                    return outs

                xijT = mlp_layer(prodT, "xij_w", "xij_b", "xijT", True)
                u1T = mlp_layer(xcnT, "xcn_w1", "xcn_b1", "u1T", True)
                u2T = mlp_layer(u1T, "xcn_w2", "xcn_b2", "u2T", False)
                zT = []
                for h in range(2):
                    zb = pC.tile([P, W], bf16, tag=f"zb{h}", name=f"zb{h}_{grp}")
                    nc.vector.tensor_tensor(
                        out=zb[:], in0=u2T[h][:],
                        in1=beta_sb[:, 0:1].to_broadcast([P, W]), op=MUL,
                    )
                    zt = pC.tile([P, W], bf16, tag=f"zT{h}", name=f"zT{h}_{grp}")
                    nc.vector.tensor_tensor(
                        out=zt[:], in0=zb[:], in1=xijT[h][:], op=ADD
                    )
                    zT.append(zt)
                vT = mlp_layer(zT, "lin_w1", "lin_b1", "vT", False)
                pso = psO.tile([1, W], f32, tag="pso", name=f"pso{grp}")
                nc.tensor.matmul(
                    pso[:], lw2_sb[0][:], vT[0][:], start=True, stop=False
                )
                nc.tensor.matmul(
                    pso[:], lw2_sb[1][:], vT[1][:], start=False, stop=True
                )
                nc.scalar.activation(
                    out_row[0:1, grp * W:(grp + 1) * W], pso[:],
                    Ident, bias=lb2_sb[0:1, 0:1],
                )

            # ---- software-pipelined emission ----
            with tc.tile_pool(name="psB", bufs=1, space="PSUM") as psB:
                ps_map = {}

                def open_half(bh):
                    for b in range(bh * CGRP, (bh + 1) * CGRP):
                        ps_map[b] = psB.tile(
                            [P, C], f32, tag=f"psxcn{b % CGRP}",
                            name=f"psxcn{b}")

                def b_loads(bh, s):
                    for b in range(bh * CGRP, (bh + 1) * CGRP):
                        stage_b_load(b, s)

                def b_mms(bh, s):
                    for b in range(bh * CGRP, (bh + 1) * CGRP):
                        stage_b_mms(b, s, ps_map[b])

                with tc.tile_pool(name="stA", bufs=3) as pA, \
                     tc.tile_pool(name="psA", bufs=4, space="PSUM") as psA:
                    open_half(0)
                    b_loads(0, 0)
                    for g in range(8):
                        stage_a_group(g, pA, psA)
                    b_mms(0, 0)
                    stage_c_prod(0)
                    b_loads(0, 1)
                    for g in range(8, 16):
                        stage_a_group(g, pA, psA)
                with tc.tile_pool(name="psC", bufs=2, space="PSUM") as psC, \
                     tc.tile_pool(name="psO", bufs=1, space="PSUM") as psO:
                    b_mms(0, 1)
                    stage_c_prod(1)
                    for b in range(CGRP):
                        stage_b_finish(b, ps_map[b])
                    open_half(1)
                    b_loads(1, 0)
                    b_mms(1, 0)
                    b_loads(1, 1)
                    stage_c(0, psC, psO)
                    b_mms(1, 1)
                    for b in range(CGRP, 2 * CGRP):
                        stage_b_finish(b, ps_map[b])
                    stage_c(1, psC, psO)

            nc.sync.dma_start(out=out_d[:, :], in_=out_row[0:1, :])
            if DEBUG_DUMPS:
                nc.sync.dma_start(out=dbg["h_all"][:, :], in_=h_all[:])

    return _split_multi_waits(nc) if split_waits else nc


def kernel(**inputs):
    from concourse.bass_utils import run_bass_kernel_spmd

    if "nc" not in _CACHE:
        _CACHE["nc"] = _build()
    nc = _CACHE["nc"]

    x = np.ascontiguousarray(inputs["x"], dtype=np.float32)
    adj8 = np.ascontiguousarray(inputs["adj"]).astype(ml_dtypes.float8_e4m3)
    tar = np.asarray(inputs["tar_ei"]).astype(np.int32)

    def btile(b):
        return np.ascontiguousarray(np.asarray(b, dtype=np.float32).reshape(2, P).T)

    common = {
        "x": x.astype(ml_dtypes.bfloat16),
        "xT": np.ascontiguousarray(x.T).astype(ml_dtypes.bfloat16),
        "adj": adj8,
        "beta_bc": np.full((P, 1), np.asarray(inputs["beta"]).reshape(-1)[0],
                           dtype=np.float32),
        "lin_w2": np.ascontiguousarray(inputs["lin_w2"]).astype(ml_dtypes.bfloat16),
        "lin_b2": np.asarray(inputs["lin_b2"], dtype=np.float32).reshape(1, 1),
    }
    for n in ("xlin_w1", "xlin_w2"):
        common[n] = np.ascontiguousarray(inputs[n]).astype(ml_dtypes.bfloat16)
    for n in ("xcn_w1", "xcn_w2", "xij_w", "lin_w1"):
        common[n] = np.ascontiguousarray(inputs[n]).astype(ml_dtypes.bfloat16)
    common["ball"] = np.ascontiguousarray(np.concatenate(
        [btile(inputs[n]) for n in
         ("xlin_b1", "xlin_b2", "xcn_b1", "xcn_b2", "xij_b", "lin_b1")],
        axis=1))

    in_maps = []
    for c in range(NCORES):
        m = dict(common)
        m["idx"] = np.ascontiguousarray(tar[:, c * EL:(c + 1) * EL])
        in_maps.append(m)

    res = run_bass_kernel_spmd(
        nc, in_maps, core_ids=list(range(NCORES)), trace=TRACE
    )
    global LAST_RESULT
    LAST_RESULT = res
    out = np.concatenate(
        [res.results[c]["out"].reshape(EL, 1) for c in range(NCORES)], axis=0
    )
    return out.astype(np.float32)



# revision 47
# speedup vs baseline: 1.3309x; 1.3309x over previous
"""CNLinkPredictor Trainium2 kernel — packed-adjacency redesign.

Edge-sharded across 8 NeuronCores (1024 edges each); x, adj, weights
replicated. Key ideas vs the first working version:

  * Adjacency is bit-packed on the host, 2 k-entries per fp8 byte:
    adj2[n, kp] = A[n, 2kp] + 2*A[n, 2kp+1]  (values 0..3, fp8-exact).
    Per edge we gather HALF the bytes (4096 instead of 8192).
  * q = adj2[i] * adj2[j] takes values in {0,1,2,3,4,6,9} (bf16-exact) and
    encodes BOTH common-neighbor bits:
      cn[e, 2kp]   = q mod 2   (only the a0*b0 term is odd)
      cn[e, 2kp+1] = q >= 4    (the 4*a1*b1 term dominates: q<4 iff a1*b1=0)
    So only ONE [128, 4096] tensor goes through the expensive DMA
    transpose (half the transpose traffic of unpacked cn), and the two
    extractions run post-transpose as 4x-mode tensor_scalar ops.
  * The host permutes node rows x' = [x[0::2]; x[1::2]] so the packed
    k-pair (2kp, 2kp+1) maps to h'-rows (kp, 4096+kp): the even/odd cn
    planes pair with contiguous h tiles and no on-device shuffling is
    needed. Edge-endpoint indices are translated on the host to match.
  * Stage-B matmuls use h tiles as the STATIONARY operand and the cn
    planes as MOVING (2 blocks = 256 edges per matmul), producing xcnT
    directly in channel-major layout — the xcn transposes of stage C
    disappear.

Hardware pitfalls carried over from the previous version:
  - one sync-wait per instruction (tile patch + wait splitting below);
  - steady-state DMA <= 2 bytes/element (fp8/bf16 only; f32 loads up
    front, the single f32 store at the very end);
  - xbar transpose destinations must be contiguous 2-D spans.
"""

import numpy as np
import ml_dtypes

N = 8192
C = 256
E = 8192
NCORES = 8
EL = E // NCORES          # edges per core (1024)
P = 128
NB = EL // P              # edge blocks per core (8)
KP = N // 2               # packed k width (4096)
NKT = KP // P             # qT k-tiles per block (32)
AGRP = 512                # stage-A node group
CGRP = 4                  # stage-C blocks per group (512 edges)

_CACHE = {}
TRACE = False
LAST_RESULT = None


def _apply_tile_patch():
    """Split the Tile tail-drain's multi-sem wait onto individual SP nops."""
    from concourse.tile import TileContext
    from concourse.vector_clock import ScopedClock

    if getattr(TileContext, "_drain_patched", False):
        return

    def _patched(self, tick_clock, wait_clock):
        nc = self.nc
        collector = nc.sync.nop()
        wait_clock.add_sem_waits(
            collector.ins, ScopedClock({None: tick_clock.global_clock})
        )
        si = collector.ins.sync_info
        waits = list(si.on_wait) if si is not None and si.on_wait else []
        if si is not None and len(waits) > 1:
            name_to_handle = {h.name: h for h in self.sems.allocated().values()}
            si.on_wait = [waits[0]]
            for w in waits[1:]:
                op = {
                    "sem-ge-imm": "sem-ge",
                    "sem-eq-imm": "sem-eq",
                    "sem-le-imm": "sem-le",
                }.get(str(w.wait_mode), "sem-ge")
                nc.sync.nop().wait_op(name_to_handle[w.ant_name], w.wait_value, op)
        nc.sync.drain()
        nc.all_engine_barrier()
        assert self.sems is not None
        popped = nc._tile_sem_poison_stack.pop()
        assert popped is self._sem_poison
        nc.clear_and_free_semaphores(list(self.sems.allocated().values()))
        nc.all_engine_barrier()

    TileContext._drain_and_barrier = _patched
    TileContext._drain_patched = True


def _split_multi_waits(nc):
    """Hoist extra sync-waits onto same-engine NoOps (sequential waits ==
    ANDed waits); this walrus build allows one wait per instruction."""
    import concourse.mybir as mybir

    cnt = 0
    for fn in nc.m.functions:
        for bb in fn.blocks:
            out = []
            for inst in bb.instructions:
                si = getattr(inst, "sync_info", None)
                waits = list(si.on_wait) if si is not None and si.on_wait else []
                if len(waits) > 1:
                    for w in waits[:-1]:
                        nop = mybir.InstNoOp(name=f"ws-{cnt}", ins=[], outs=[])
                        cnt += 1
                        nop.engine = inst.engine
                        nop.sync_info = mybir.SyncInfo(on_wait=[w], on_update=[])
                        out.append(nop)
                    si.on_wait = [waits[-1]]
                out.append(inst)
            bb.instructions = out
    return nc


def _build(split_waits=True):
    import concourse.bass as bass
    import concourse.mybir as mybir
    from concourse.tile import TileContext

    _apply_tile_patch()

    f32 = mybir.dt.float32
    bf16 = mybir.dt.bfloat16
    fp8 = mybir.dt.float8e4
    u16 = mybir.dt.uint16
    i32 = mybir.dt.int32
    Relu = mybir.ActivationFunctionType.Relu
    Ident = mybir.ActivationFunctionType.Identity
    MUL = mybir.AluOpType.mult
    ADD = mybir.AluOpType.add
    GE = mybir.AluOpType.is_ge
    AND = mybir.AluOpType.bitwise_and

    nc = bass.Bass(num_swdge_queues=4)

    xT_d = nc.dram_tensor("xT", [C, N], bf16, kind="ExternalInput")
    xnat_d = nc.dram_tensor("xnat", [N, C], bf16, kind="ExternalInput")
    adj2_d = nc.dram_tensor("adj2", [N, KP // 2], u16, kind="ExternalInput")
    # [P, 2*NB (adj i,j interleaved) + 2*NB (xnat i,j interleaved)]
    idx_d = nc.dram_tensor("idx", [P, 4 * NB], i32, kind="ExternalInput")
    wnames = ["xlin_w1", "xlin_w2", "xcn_w1", "xcn_w2", "xij_w", "lin_w1"]
    bnames = ["xlin_b1", "xlin_b2", "xcn_b1", "xcn_b2", "xij_b", "lin_b1"]
    # one blob for all bf16 weights (6 x [P, 512] + lin_w2 [P, 2]), one for
    # all f32 scalars (biases 12 + beta 1 + lin_b2 1): 3 const DMAs total
    wall_d = nc.dram_tensor("wall", [P, 6 * 2 * C + 2], bf16,
                            kind="ExternalInput")
    ball_d = nc.dram_tensor("ball", [P, 2 * len(bnames) + 2], f32,
                            kind="ExternalInput")
    out_d = nc.dram_tensor("out", [1, EL], f32, kind="ExternalOutput")

    _swq = [0]

    def _rr(inst):
        q = _swq[0] % 4
        _swq[0] += 1
        if q:
            inst.ins.queue = f"qPoolDynamic{q}"
        return inst

    with TileContext(nc) as tc:
        with (
            tc.tile_pool(name="const", bufs=1) as pK,
            tc.tile_pool(name="hpool", bufs=1) as pH,
            tc.tile_pool(name="adj", bufs=2) as pAdj,
            tc.tile_pool(name="qp", bufs=1) as pQ,
            tc.tile_pool(name="xtld", bufs=8) as pXT,
            tc.tile_pool(name="qtp", bufs=2) as pQT,
            tc.tile_pool(name="t16", bufs=1) as pT16,
            tc.tile_pool(name="cnq", bufs=2) as pCN,
            tc.tile_pool(name="xcn", bufs=1) as pX,
            tc.tile_pool(name="edgeg", bufs=4) as pCg,
            tc.tile_pool(name="edget", bufs=2) as pCt,
            tc.tile_pool(name="edgem", bufs=1) as pCm,
        ):
            # ---- constants ----
            idx_sb = pK.tile([P, 4 * NB], i32, tag="idx_sb", name="idx_sb")
            nc.sync.dma_start(out=idx_sb[:], in_=idx_d[:, :])
            ia = [idx_sb[:, 2 * b:2 * b + 2] for b in range(NB)]
            ix = [idx_sb[:, 2 * NB + 2 * b:2 * NB + 2 * b + 2]
                  for b in range(NB)]

            wall = pK.tile([P, 6 * 2 * C + 2], bf16, tag="wall", name="wall")
            nc.sync.dma_start(out=wall[:], in_=wall_d[:, :])
            wA_sb, wC_sb = {}, {}
            for qi, n in enumerate(wnames):
                pair = [wall[:, qi * 2 * C:qi * 2 * C + C],
                        wall[:, qi * 2 * C + C:(qi + 1) * 2 * C]]
                (wA_sb if n.startswith("xlin_w") else wC_sb)[n] = pair
            lw2_sb = [wall[:, 12 * C:12 * C + 1], wall[:, 12 * C + 1:12 * C + 2]]
            ball = pK.tile([P, 2 * len(bnames) + 2], f32, tag="ball",
                           name="ball")
            nc.sync.dma_start(out=ball[:], in_=ball_d[:, :])
            b_sb = {}
            for qn, n in enumerate(bnames):
                b_sb[n] = ball[:, 2 * qn:2 * qn + 2]
            beta_sb = ball[:, 12:13]
            lb2_sb = ball[0:1, 13:14]

            out_row = pK.tile([1, EL], f32, tag="out_row", name="out_row")
            # h in permuted-natural layout: column = hh*N + kt*128 + c2
            # encodes h'[node' = kt*128 + p, channel = hh*128 + c2].
            h_all = pH.tile([P, 2 * N], bf16, tag="h_all", name="h_all")
            h_view = h_all[:].rearrange("p (hh kt c) -> p hh kt c", hh=2, c=P)

            # ---- stage A: h' = x' + MLP(x'), transposed, baseline-style ----
            # xT loads are batched per super-group (2 node groups = 1024
            # nodes, one DMA) and prefetched 2 super-groups ahead so the
            # ~3us per-op DMA latency never stalls the PE.
            xt_sg = {}

            def load_xt(sg):
                t = pXT.tile([P, 2 * 2 * AGRP], bf16, tag="xTsg",
                             name=f"xTsg{sg}")
                nc.sync.dma_start(
                    out=t[:].rearrange("p (k n) -> p k n", k=2),
                    in_=xT_d[:, sg * 2 * AGRP:(sg + 1) * 2 * AGRP].rearrange(
                        "(k p) n -> p k n", p=P),
                )
                xt_sg[sg] = t

            MAX = mybir.AluOpType.max

            def stage_a_group(g, pA, pAy, psA):
                t = xt_sg[g // 2]
                off = (g % 2) * AGRP
                tv = t[:].rearrange("p (k n) -> p k n", k=2)
                xT = [tv[:, h, off:off + AGRP] for h in range(2)]
                y1T = []
                for h in range(2):
                    ps = psA.tile([P, AGRP], f32, tag="psmm", name=f"psA1_{g}{h}")
                    nc.tensor.matmul(
                        ps[:], wA_sb["xlin_w1"][0][:, h * P:(h + 1) * P],
                        xT[0], start=True, stop=False,
                    )
                    nc.tensor.matmul(
                        ps[:], wA_sb["xlin_w1"][1][:, h * P:(h + 1) * P],
                        xT[1], start=False, stop=True,
                    )
                    t1 = pA.tile([P, AGRP], bf16, tag=f"y1T{h}", name=f"y1T{h}_{g}")
                    nc.scalar.activation(
                        t1[:], ps[:], Relu, bias=b_sb["xlin_b1"][:, h:h + 1]
                    )
                    y1T.append(t1)
                for h in range(2):
                    ps = psA.tile([P, AGRP], f32, tag="psmm", name=f"psA2_{g}{h}")
                    nc.tensor.matmul(
                        ps[:], wA_sb["xlin_w2"][0][:, h * P:(h + 1) * P],
                        y1T[0][:], start=True, stop=False,
                    )
                    nc.tensor.matmul(
                        ps[:], wA_sb["xlin_w2"][1][:, h * P:(h + 1) * P],
                        y1T[1][:], start=False, stop=True,
                    )
                    hT = pAy.tile([P, AGRP], bf16, tag=f"hT{h}",
                                  name=f"hT{h}_{g}")
                    nc.scalar.activation(
                        hT[:], ps[:], Relu, bias=b_sb["xlin_b2"][:, h:h + 1]
                    )
                    # residual added IN PLACE on DVE. Queue discipline:
                    # per-queue completion sems fire in order, so a compute
                    # op queued behind an async DMA inherits its late
                    # completion -- Pool (gathers) and ACT (acts only) must
                    # not carry these adds.
                    nc.vector.tensor_tensor(
                        out=hT[:], in0=hT[:], in1=xT[h], op=ADD
                    )
                    nc.sync.dma_start_transpose(
                        out=h_view[:, h,
                                   g * (AGRP // P):(g + 1) * (AGRP // P), :],
                        in_=hT[:],
                    )

            # ---- stage B pieces ----
            adj_map, q_map, qT_map = {}, {}, {}
            cnq_map = {}

            KW = KP // 2   # u16 words per adjacency row (2048)

            def gather_adj(b):
                t = pAdj.tile([P, 2 * KW], u16, tag="padj", name=f"padj{b}")
                _rr(nc.gpsimd.indirect_dma_start(
                    out=t[:, 0:KW], out_offset=None, in_=adj2_d[:, :],
                    in_offset=bass.IndirectOffsetOnAxis(
                        ap=ia[b][:, 0:1], axis=0),
                ))
                _rr(nc.gpsimd.indirect_dma_start(
                    out=t[:, KW:2 * KW], out_offset=None, in_=adj2_d[:, :],
                    in_offset=bass.IndirectOffsetOnAxis(
                        ap=ia[b][:, 1:2], axis=0),
                ))
                adj_map[b] = t

            def qprod(b):
                t = pQ.tile([P, KW], u16, tag="q", name=f"q{b}")
                pa = adj_map.pop(b)
                nc.vector.tensor_tensor(
                    out=t[:], in0=pa[:, 0:KW], in1=pa[:, KW:2 * KW], op=AND
                )
                q_map[b] = t

            def qtrans(b):
                t = pQT.tile([P, KW], u16, tag="qT", name=f"qT{b}")
                nc.sync.dma_start_transpose(
                    out=t[:].rearrange("p (kt e) -> p kt e", e=P),
                    in_=q_map.pop(b)[:],
                )
                qT_map[b] = t

            # plane r <-> k = 4*pi + r; u16 bit value and bf16 scale
            PLANES = [(2, 0.5), (1, 1.0), (512, 1.0 / 512), (256, 1.0 / 256)]
            NKT16 = KW // P    # 16 k-tiles of u16 words per block

            def extract(b):
                pr = b // 2
                if pr not in cnq_map:
                    cnq_map[pr] = [
                        pCN.tile([P, NKT16 * 2 * P], bf16, tag=f"cnq{r}",
                                 name=f"cnq{r}_{pr}")
                        for r in range(4)
                    ]
                planes = cnq_map[pr]
                qT = qT_map.pop(b)
                qv = qT[:].rearrange("p (kt e) -> p kt e", e=P)
                s = b % 2
                for r, (mask, scale) in enumerate(PLANES):
                    t = pT16.tile([P, KW], u16, tag="t16", name=f"t16_{b}_{r}")
                    nc.vector.tensor_scalar(
                        out=t[:], in0=qT[:], scalar1=mask, scalar2=None,
                        op0=AND,
                    )
                    cv = planes[r][:].rearrange(
                        "p (kt blk e) -> p kt blk e", blk=2, e=P)
                    nc.vector.tensor_scalar(
                        out=cv[:, :, s, :],
                        in0=t[:].rearrange("p (kt e) -> p kt e", e=P),
                        scalar1=scale, scalar2=None, op0=MUL,
                    )

            xcnT = pX.tile([P, 2 * EL], bf16, tag="xcnT", name="xcnT")
            # column = hh*EL + e  (channel-half on partitions)

            psB_map = {}

            def pair_mms(pr, psB):
                planes = cnq_map[pr]
                ps = [psB.tile([P, 2 * P], f32, tag=f"psb{h}",
                               name=f"psb{pr}_{h}") for h in range(2)]
                psB_map[pr] = ps
                for r in range(4):
                    cv = planes[r][:].rearrange(
                        "p (kt blk e) -> p kt (blk e)", blk=2, e=P)
                    for kt in range(NKT16):
                        for h in range(2):
                            nc.tensor.matmul(
                                ps[h][:],
                                h_view[:, h, r * NKT16 + kt, :],
                                cv[:, kt, :],
                                start=(r == 0 and kt == 0),
                                stop=(r == 3 and kt == NKT16 - 1),
                            )

            def xcn_copy(pr):
                ps = psB_map.pop(pr)
                del cnq_map[pr]
                for h in range(2):
                    nc.scalar.activation(
                        xcnT[0:P, h * EL + pr * 2 * P:h * EL + (pr + 1) * 2 * P],
                        ps[h][:], Ident,
                    )

            # ---- stage C ----
            prodT_map = {}
            xij_map = {}

            def gather_x(b):
                t = pCg.tile([P, 2 * C], bf16, tag="xij_g", name=f"xijg{b}")
                _rr(nc.gpsimd.indirect_dma_start(
                    out=t[:, 0:C], out_offset=None, in_=xnat_d[:, :],
                    in_offset=bass.IndirectOffsetOnAxis(
                        ap=ix[b][:, 0:1], axis=0),
                ))
                _rr(nc.gpsimd.indirect_dma_start(
                    out=t[:, C:2 * C], out_offset=None, in_=xnat_d[:, :],
                    in_offset=bass.IndirectOffsetOnAxis(
                        ap=ix[b][:, 1:2], axis=0),
                ))
                xij_map[b] = t

            def prod(b):
                t = pCg.tile([P, C], bf16, tag="prod", name=f"prod{b}")
                g = xij_map.pop(b)
                nc.vector.tensor_tensor(
                    out=t[:], in0=g[:, 0:C], in1=g[:, C:2 * C], op=MUL
                )
                return t

            def prodT(grp, tiles):
                W = CGRP * P
                t = pCt.tile([P, 2 * W], bf16, tag="prodT", name=f"prodT{grp}")
                tv = t[:].rearrange("p (blk hh e) -> p blk hh e", blk=CGRP, e=P)
                prodT_map[grp] = t
                for t2, pt in enumerate(tiles):
                    nc.sync.dma_start_transpose(out=tv[:, t2, :, :], in_=pt[:])

            def stage_c(grp, psC, psO):
                W = CGRP * P  # 512 edges
                r0 = xcnT[0:P, grp * W:(grp + 1) * W]
                r1 = xcnT[0:P, EL + grp * W:EL + (grp + 1) * W]
                pT = prodT_map.pop(grp)

                def mlp_layer(rhs2, wname, bname, outtag, packed,
                              tagas=None):
                    tagas = tagas or outtag
                    outs = []
                    for h in range(2):
                        ps = psC.tile([P, W], f32, tag="psc",
                                      name=f"psc_{grp}_{outtag}{h}")
                        if packed:
                            rhs_v = rhs2[:].rearrange(
                                "p (blk hh e) -> p blk hh e", blk=CGRP, e=P)
                            rr0, rr1 = rhs_v[:, :, 0, :], rhs_v[:, :, 1, :]
                        else:
                            rr0, rr1 = rhs2[0], rhs2[1]
                        nc.tensor.matmul(
                            ps[:], wC_sb[wname][0][:, h * P:(h + 1) * P],
                            rr0, start=True, stop=False,
                        )
                        nc.tensor.matmul(
                            ps[:], wC_sb[wname][1][:, h * P:(h + 1) * P],
                            rr1, start=False, stop=True,
                        )
                        t = pCm.tile([P, W], bf16, tag=f"{tagas}{h}",
                                    name=f"{outtag}{h}_{grp}")
                        nc.scalar.activation(
                            t[:], ps[:], Relu, bias=b_sb[bname][:, h:h + 1]
                        )
                        outs.append(t)
                    return outs

                def aps(ts):
                    return [t[:] for t in ts]

                xijT = mlp_layer(pT, "xij_w", "xij_b", "xijT", True)
                u1T = mlp_layer((r0, r1), "xcn_w1", "xcn_b1", "u1T", False)
                u2T = mlp_layer(aps(u1T), "xcn_w2", "xcn_b2", "u2T", False)
                zT = []
                for h in range(2):
                    zb = pCm.tile([P, W], bf16, tag=f"u1T{h}", name=f"zb{h}_{grp}")
                    nc.vector.tensor_tensor(
                        out=zb[:], in0=u2T[h][:],
                        in1=beta_sb.to_broadcast([P, W]), op=MUL,
                    )
                    zt = pCm.tile([P, W], bf16, tag=f"zT{h}", name=f"zT{h}_{grp}")
                    nc.vector.tensor_tensor(
                        out=zt[:], in0=zb[:], in1=xijT[h][:], op=ADD
                    )
                    zT.append(zt)
                vT = mlp_layer(aps(zT), "lin_w1", "lin_b1", "vT", False,
                               tagas="u2T")
                pso = psO.tile([1, W], f32, tag="pso", name=f"pso{grp}")
                nc.tensor.matmul(
                    pso[:], lw2_sb[0][:], vT[0][:], start=True, stop=False
                )
                nc.tensor.matmul(
                    pso[:], lw2_sb[1][:], vT[1][:], start=False, stop=True
                )
                nc.scalar.activation(
                    out_row[0:1, grp * W:(grp + 1) * W], pso[:],
                    Ident, bias=lb2_sb,
                )

            # ---- phased emission ----
            # Phase 1: stage A completely clean -- no B-chain op shares any
            # queue stage A depends on, so the A loop paces at the ACT rate.
            # Phase 2: B chain (gathers/AND/transpose/extract) pipelined with
            # the pair matmuls and stage C.
            with tc.tile_pool(name="psB", bufs=2, space="PSUM") as psB:
                with tc.tile_pool(name="stA", bufs=3) as pA, \
                     tc.tile_pool(name="stAy", bufs=4) as pAy, \
                     tc.tile_pool(name="psA", bufs=4, space="PSUM") as psA:
                    for _sg in range(8):
                        load_xt(_sg)
                    gather_x(0); gather_x(1); gather_x(2); gather_x(3)
                    gather_x(4); gather_x(5); gather_x(6); gather_x(7)
                    stage_a_group(0, pA, pAy, psA)
                    stage_a_group(1, pA, pAy, psA)
                    stage_a_group(2, pA, pAy, psA)
                    stage_a_group(3, pA, pAy, psA)
                    stage_a_group(4, pA, pAy, psA)
                    stage_a_group(5, pA, pAy, psA)
                    stage_a_group(6, pA, pAy, psA)
                    stage_a_group(7, pA, pAy, psA)
                    stage_a_group(8, pA, pAy, psA)
                    stage_a_group(9, pA, pAy, psA)
                    stage_a_group(10, pA, pAy, psA)
                    stage_a_group(11, pA, pAy, psA)
                    stage_a_group(12, pA, pAy, psA)
                    stage_a_group(13, pA, pAy, psA)
                    stage_a_group(14, pA, pAy, psA)
                    stage_a_group(15, pA, pAy, psA)
                    pt0 = [prod(b) for b in range(CGRP)]
                    prodT(0, pt0)
                    pt1 = [prod(b) for b in range(CGRP, 2 * CGRP)]
                    prodT(1, pt1)
                # Phase 2
                gather_adj(0); gather_adj(1)
                qprod(0)
                gather_adj(2)
                qtrans(0)
                qprod(1)
                extract(0)
                gather_adj(3)
                qtrans(1)
                qprod(2)
                extract(1)
                with tc.tile_pool(name="psC", bufs=2, space="PSUM") as psC, \
                     tc.tile_pool(name="psO", bufs=1, space="PSUM") as psO:
                    pair_mms(0, psB)
                    gather_adj(4)
                    qtrans(2)
                    qprod(3)
                    extract(2)
                    gather_adj(5)
                    qtrans(3)
                    qprod(4)
                    extract(3)
                    xcn_copy(0)
                    pair_mms(1, psB)
                    gather_adj(6)
                    qtrans(4)
                    qprod(5)
                    extract(4)
                    gather_adj(7)
                    qtrans(5)
                    qprod(6)
                    extract(5)
                    xcn_copy(1)
                    pair_mms(2, psB)
                    qtrans(6)
                    qprod(7)
                    extract(6)
                    qtrans(7)
                    extract(7)
                    xcn_copy(2)
                    stage_c(0, psC, psO)
                    pair_mms(3, psB)
                    xcn_copy(3)
                    stage_c(1, psC, psO)

            nc.sync.dma_start(out=out_d[:, :], in_=out_row[0:1, :])

    return _split_multi_waits(nc) if split_waits else nc


def kernel(**inputs):
    from concourse.bass_utils import run_bass_kernel_spmd

    if "nc" not in _CACHE:
        _CACHE["nc"] = _build()
    nc = _CACHE["nc"]

    x = np.ascontiguousarray(inputs["x"], dtype=np.float32)
    adj = np.asarray(inputs["adj"], dtype=np.float32)
    tar = np.asarray(inputs["tar_ei"]).astype(np.int64)

    # byte kp = 2*A[:, 2kp] + A[:, 2kp+1]; u16 word = byte pair (LE), so
    # word pi encodes k = 4pi..4pi+3 at bit values (2, 1, 512, 256)
    ab = (2.0 * adj[:, 0::2] + adj[:, 1::2]).astype(np.uint8)
    adj2 = np.ascontiguousarray(ab).view(np.uint16)
    # node permutation: k = 4pi + r lives in quarter r, row pi
    xp = np.concatenate([x[0::4], x[1::4], x[2::4], x[3::4]], axis=0)

    def btile(b):
        return np.ascontiguousarray(
            np.asarray(b, dtype=np.float32).reshape(2, P).T)

    def wtile(w):
        # [256, M] -> [128, 2*M] with the two k-halves side by side
        w = np.asarray(w, dtype=np.float32)
        return w.reshape(2, P, -1).transpose(1, 0, 2).reshape(P, -1)

    wall = np.concatenate(
        [wtile(inputs[n]) for n in
         ("xlin_w1", "xlin_w2", "xcn_w1", "xcn_w2", "xij_w", "lin_w1")]
        + [wtile(inputs["lin_w2"])], axis=1)
    ball = np.concatenate(
        [btile(inputs[n]) for n in
         ("xlin_b1", "xlin_b2", "xcn_b1", "xcn_b2", "xij_b", "lin_b1")]
        + [np.full((P, 1), np.asarray(inputs["beta"]).reshape(-1)[0],
                   dtype=np.float32),
           np.full((P, 1), np.asarray(inputs["lin_b2"]).reshape(-1)[0],
                   dtype=np.float32)], axis=1)

    common = {
        "xnat": np.ascontiguousarray(xp).astype(ml_dtypes.bfloat16),
        "xT": np.ascontiguousarray(xp.T).astype(ml_dtypes.bfloat16),
        "adj2": np.ascontiguousarray(adj2),
        "wall": np.ascontiguousarray(wall).astype(ml_dtypes.bfloat16),
        "ball": np.ascontiguousarray(ball),
    }

    in_maps = []
    for c in range(NCORES):
        t = tar[:, c * EL:(c + 1) * EL]              # [2, 1024]
        # adj row indices (original node ids), interleaved (i,j) per block
        ta = t.reshape(2, NB, P)                     # [t, b, p]
        idx_adj = np.transpose(ta, (2, 1, 0)).reshape(P, 2 * NB)
        # xnat row indices (permuted positions)
        tp = (t >> 2) + (N // 4) * (t & 3)
        tx = tp.reshape(2, NB, P)
        idx_x = np.transpose(tx, (2, 1, 0)).reshape(P, 2 * NB)
        m = dict(common)
        m["idx"] = np.ascontiguousarray(
            np.concatenate([idx_adj, idx_x], axis=1).astype(np.int32))
        in_maps.append(m)

    res = run_bass_kernel_spmd(
        nc, in_maps, core_ids=list(range(NCORES)), trace=TRACE
    )
    global LAST_RESULT
    LAST_RESULT = res
    out = np.concatenate(
        [res.results[c]["out"].reshape(EL, 1) for c in range(NCORES)], axis=0
    )
    return out.astype(np.float32)


# revision 57
# speedup vs baseline: 1.5944x; 1.1980x over previous
"""CNLinkPredictor Trainium2 kernel — packed-adjacency redesign.

Edge-sharded across 8 NeuronCores (1024 edges each); x, adj, weights
replicated. Key ideas vs the first working version:

  * Adjacency is bit-packed on the host, 2 k-entries per fp8 byte:
    adj2[n, kp] = A[n, 2kp] + 2*A[n, 2kp+1]  (values 0..3, fp8-exact).
    Per edge we gather HALF the bytes (4096 instead of 8192).
  * q = adj2[i] * adj2[j] takes values in {0,1,2,3,4,6,9} (bf16-exact) and
    encodes BOTH common-neighbor bits:
      cn[e, 2kp]   = q mod 2   (only the a0*b0 term is odd)
      cn[e, 2kp+1] = q >= 4    (the 4*a1*b1 term dominates: q<4 iff a1*b1=0)
    So only ONE [128, 4096] tensor goes through the expensive DMA
    transpose (half the transpose traffic of unpacked cn), and the two
    extractions run post-transpose as 4x-mode tensor_scalar ops.
  * The host permutes node rows x' = [x[0::2]; x[1::2]] so the packed
    k-pair (2kp, 2kp+1) maps to h'-rows (kp, 4096+kp): the even/odd cn
    planes pair with contiguous h tiles and no on-device shuffling is
    needed. Edge-endpoint indices are translated on the host to match.
  * Stage-B matmuls use h tiles as the STATIONARY operand and the cn
    planes as MOVING (2 blocks = 256 edges per matmul), producing xcnT
    directly in channel-major layout — the xcn transposes of stage C
    disappear.

Hardware pitfalls carried over from the previous version:
  - one sync-wait per instruction (tile patch + wait splitting below);
  - steady-state DMA <= 2 bytes/element (fp8/bf16 only; f32 loads up
    front, the single f32 store at the very end);
  - xbar transpose destinations must be contiguous 2-D spans.
"""

import numpy as np
import ml_dtypes

N = 8192
C = 256
E = 8192
NCORES = 8
EL = E // NCORES          # edges per core (1024)
P = 128
NB = EL // P              # edge blocks per core (8)
KP = N // 2               # packed k width (4096)
NKT = KP // P             # qT k-tiles per block (32)
AGRP = 512                # stage-A node group
CGRP = 4                  # stage-C blocks per group (512 edges)

_CACHE = {}
TRACE = False
LAST_RESULT = None


def _apply_tile_patch():
    """Split the Tile tail-drain's multi-sem wait onto individual SP nops."""
    from concourse.tile import TileContext
    from concourse.vector_clock import ScopedClock

    if getattr(TileContext, "_drain_patched", False):
        return

    def _patched(self, tick_clock, wait_clock):
        nc = self.nc
        collector = nc.sync.nop()
        wait_clock.add_sem_waits(
            collector.ins, ScopedClock({None: tick_clock.global_clock})
        )
        si = collector.ins.sync_info
        waits = list(si.on_wait) if si is not None and si.on_wait else []
        if si is not None and len(waits) > 1:
            name_to_handle = {h.name: h for h in self.sems.allocated().values()}
            si.on_wait = [waits[0]]
            for w in waits[1:]:
                op = {
                    "sem-ge-imm": "sem-ge",
                    "sem-eq-imm": "sem-eq",
                    "sem-le-imm": "sem-le",
                }.get(str(w.wait_mode), "sem-ge")
                nc.sync.nop().wait_op(name_to_handle[w.ant_name], w.wait_value, op)
        nc.sync.drain()
        nc.all_engine_barrier()
        assert self.sems is not None
        popped = nc._tile_sem_poison_stack.pop()
        assert popped is self._sem_poison
        nc.clear_and_free_semaphores(list(self.sems.allocated().values()))
        nc.all_engine_barrier()

    TileContext._drain_and_barrier = _patched
    TileContext._drain_patched = True


def _split_multi_waits(nc):
    """Hoist extra sync-waits onto same-engine NoOps (sequential waits ==
    ANDed waits); this walrus build allows one wait per instruction."""
    import concourse.mybir as mybir

    cnt = 0
    for fn in nc.m.functions:
        for bb in fn.blocks:
            out = []
            for inst in bb.instructions:
                si = getattr(inst, "sync_info", None)
                waits = list(si.on_wait) if si is not None and si.on_wait else []
                if len(waits) > 1:
                    for w in waits[:-1]:
                        nop = mybir.InstNoOp(name=f"ws-{cnt}", ins=[], outs=[])
                        cnt += 1
                        nop.engine = inst.engine
                        nop.sync_info = mybir.SyncInfo(on_wait=[w], on_update=[])
                        out.append(nop)
                    si.on_wait = [waits[-1]]
                out.append(inst)
            bb.instructions = out
    return nc


def _build(split_waits=True):
    import concourse.bass as bass
    import concourse.mybir as mybir
    from concourse.tile import TileContext

    _apply_tile_patch()

    f32 = mybir.dt.float32
    bf16 = mybir.dt.bfloat16
    fp8 = mybir.dt.float8e4
    u16 = mybir.dt.uint16
    i32 = mybir.dt.int32
    Relu = mybir.ActivationFunctionType.Relu
    Ident = mybir.ActivationFunctionType.Identity
    MUL = mybir.AluOpType.mult
    ADD = mybir.AluOpType.add
    GE = mybir.AluOpType.is_ge
    AND = mybir.AluOpType.bitwise_and

    nc = bass.Bass(num_swdge_queues=4)

    xT_d = nc.dram_tensor("xT", [C, N], bf16, kind="ExternalInput")
    xnat_d = nc.dram_tensor("xnat", [N, C], bf16, kind="ExternalInput")
    adj2_d = nc.dram_tensor("adj2", [N, KP // 2], u16, kind="ExternalInput")
    # [P, 2*NB (adj i,j interleaved) + 2*NB (xnat i,j interleaved)]
    idx_d = nc.dram_tensor("idx", [P, 4 * NB], i32, kind="ExternalInput")
    wnames = ["xlin_w1", "xlin_w2", "xcn_w1", "xcn_w2", "xij_w", "lin_w1"]
    bnames = ["xlin_b1", "xlin_b2", "xcn_b1", "xcn_b2", "xij_b", "lin_b1"]
    # one blob for all bf16 weights (6 x [P, 512] + lin_w2 [P, 2]), one for
    # all f32 scalars (biases 12 + beta 1 + lin_b2 1): 3 const DMAs total
    wall_d = nc.dram_tensor("wall", [P, 6 * 2 * C + 2], bf16,
                            kind="ExternalInput")
    ball_d = nc.dram_tensor("ball", [P, 2 * len(bnames) + 2], f32,
                            kind="ExternalInput")
    out_d = nc.dram_tensor("out", [1, EL], f32, kind="ExternalOutput")

    _swq = [0]

    def _rr(inst):
        q = _swq[0] % 4
        _swq[0] += 1
        if q:
            inst.ins.queue = f"qPoolDynamic{q}"
        return inst

    with TileContext(nc) as tc:
        with (
            tc.tile_pool(name="const", bufs=1) as pK,
            tc.tile_pool(name="hpool", bufs=1) as pH,
            tc.tile_pool(name="adj", bufs=2) as pAdj,
            tc.tile_pool(name="qp", bufs=1) as pQ,
            tc.tile_pool(name="xtld", bufs=8) as pXT,
            tc.tile_pool(name="qtp", bufs=2) as pQT,
            tc.tile_pool(name="t16", bufs=1) as pT16,
            tc.tile_pool(name="cnq", bufs=2) as pCN,
            tc.tile_pool(name="xcn", bufs=1) as pX,
            tc.tile_pool(name="edgeg", bufs=4) as pCg,
            tc.tile_pool(name="edget", bufs=2) as pCt,
            tc.tile_pool(name="edgem", bufs=1) as pCm,
        ):
            # ---- constants ----
            idx_sb = pK.tile([P, 4 * NB], i32, tag="idx_sb", name="idx_sb")
            nc.sync.dma_start(out=idx_sb[:], in_=idx_d[:, :])
            ia = [idx_sb[:, 2 * b:2 * b + 2] for b in range(NB)]
            ix = [idx_sb[:, 2 * NB + 2 * b:2 * NB + 2 * b + 2]
                  for b in range(NB)]

            wall = pK.tile([P, 6 * 2 * C + 2], bf16, tag="wall", name="wall")
            nc.sync.dma_start(out=wall[:], in_=wall_d[:, :])
            wA_sb, wC_sb = {}, {}
            for qi, n in enumerate(wnames):
                pair = [wall[:, qi * 2 * C:qi * 2 * C + C],
                        wall[:, qi * 2 * C + C:(qi + 1) * 2 * C]]
                (wA_sb if n.startswith("xlin_w") else wC_sb)[n] = pair
            lw2_sb = [wall[:, 12 * C:12 * C + 1], wall[:, 12 * C + 1:12 * C + 2]]
            ball = pK.tile([P, 2 * len(bnames) + 2], f32, tag="ball",
                           name="ball")
            nc.sync.dma_start(out=ball[:], in_=ball_d[:, :])
            b_sb = {}
            for qn, n in enumerate(bnames):
                b_sb[n] = ball[:, 2 * qn:2 * qn + 2]
            beta_sb = ball[:, 12:13]
            lb2_sb = ball[0:1, 13:14]

            out_row = pK.tile([1, EL], f32, tag="out_row", name="out_row")
            # h in permuted-natural layout: column = hh*N + kt*128 + c2
            # encodes h'[node' = kt*128 + p, channel = hh*128 + c2].
            h_all = pH.tile([P, 2 * N], bf16, tag="h_all", name="h_all")
            h_view = h_all[:].rearrange("p (hh kt c) -> p hh kt c", hh=2, c=P)

            # ---- stage A: h' = x' + MLP(x'), transposed, baseline-style ----
            # xT loads are batched per super-group (2 node groups = 1024
            # nodes, one DMA) and prefetched 2 super-groups ahead so the
            # ~3us per-op DMA latency never stalls the PE.
            xt_sg = {}

            def load_xt(sg):
                t = pXT.tile([P, 2 * 2 * AGRP], bf16, tag="xTsg",
                             name=f"xTsg{sg}")
                nc.sync.dma_start(
                    out=t[:].rearrange("p (k n) -> p k n", k=2),
                    in_=xT_d[:, sg * 2 * AGRP:(sg + 1) * 2 * AGRP].rearrange(
                        "(k p) n -> p k n", p=P),
                )
                xt_sg[sg] = t

            MAX = mybir.AluOpType.max

            def stage_a_group(g, pA, pAy, psA):
                t = xt_sg[g // 2]
                off = (g % 2) * AGRP
                tv = t[:].rearrange("p (k n) -> p k n", k=2)
                xT = [tv[:, h, off:off + AGRP] for h in range(2)]
                y1T = []
                for h in range(2):
                    ps = psA.tile([P, AGRP], f32, tag="psmm", name=f"psA1_{g}{h}")
                    nc.tensor.matmul(
                        ps[:], wA_sb["xlin_w1"][0][:, h * P:(h + 1) * P],
                        xT[0], start=True, stop=False,
                    )
                    nc.tensor.matmul(
                        ps[:], wA_sb["xlin_w1"][1][:, h * P:(h + 1) * P],
                        xT[1], start=False, stop=True,
                    )
                    t1 = pA.tile([P, AGRP], bf16, tag=f"y1T{h}", name=f"y1T{h}_{g}")
                    nc.scalar.activation(
                        t1[:], ps[:], Relu, bias=b_sb["xlin_b1"][:, h:h + 1]
                    )
                    y1T.append(t1)
                for h in range(2):
                    ps = psA.tile([P, AGRP], f32, tag="psmm", name=f"psA2_{g}{h}")
                    nc.tensor.matmul(
                        ps[:], wA_sb["xlin_w2"][0][:, h * P:(h + 1) * P],
                        y1T[0][:], start=True, stop=False,
                    )
                    nc.tensor.matmul(
                        ps[:], wA_sb["xlin_w2"][1][:, h * P:(h + 1) * P],
                        y1T[1][:], start=False, stop=True,
                    )
                    hT = pAy.tile([P, AGRP], bf16, tag=f"hT{h}",
                                  name=f"hT{h}_{g}")
                    nc.scalar.activation(
                        hT[:], ps[:], Relu, bias=b_sb["xlin_b2"][:, h:h + 1]
                    )
                    # residual added IN PLACE on DVE. Queue discipline:
                    # per-queue completion sems fire in order, so a compute
                    # op queued behind an async DMA inherits its late
                    # completion -- Pool (gathers) and ACT (acts only) must
                    # not carry these adds.
                    nc.vector.tensor_tensor(
                        out=hT[:], in0=hT[:], in1=xT[h], op=ADD
                    )
                    nc.sync.dma_start_transpose(
                        out=h_view[:, h,
                                   g * (AGRP // P):(g + 1) * (AGRP // P), :],
                        in_=hT[:],
                    )

            # ---- stage B pieces ----
            adj_map, q_map, qT_map = {}, {}, {}
            cnq_map = {}

            KW = KP // 2   # u16 words per adjacency row (2048)

            def gather_adj(pr):
                # one tile per PAIR of edge blocks: [t(i,j), blk, KW]
                t = pAdj.tile([P, 2 * 2 * KW], u16, tag="padj",
                              name=f"padj{pr}")
                tv = t[:].rearrange("p (t blk k) -> p t blk k", t=2, blk=2)
                for s in range(2):
                    b = 2 * pr + s
                    _rr(nc.gpsimd.indirect_dma_start(
                        out=tv[:, 0, s, :], out_offset=None, in_=adj2_d[:, :],
                        in_offset=bass.IndirectOffsetOnAxis(
                            ap=ia[b][:, 0:1], axis=0),
                    ))
                    _rr(nc.gpsimd.indirect_dma_start(
                        out=tv[:, 1, s, :], out_offset=None, in_=adj2_d[:, :],
                        in_offset=bass.IndirectOffsetOnAxis(
                            ap=ia[b][:, 1:2], axis=0),
                    ))
                adj_map[pr] = t

            def qprod(pr):
                # AND both blocks of the pair in one [128, 4096] op
                t = pQ.tile([P, 2 * KW], u16, tag="q", name=f"q{pr}")
                pa = adj_map.pop(pr)
                nc.vector.tensor_tensor(
                    out=t[:], in0=pa[:, 0:2 * KW], in1=pa[:, 2 * KW:4 * KW],
                    op=AND,
                )
                q_map[pr] = t

            def qtrans(pr):
                # one transpose per pair; output k-tile order = (blk, kt16)
                t = pQT.tile([P, 2 * KW], u16, tag="qT", name=f"qT{pr}")
                nc.sync.dma_start_transpose(
                    out=t[:].rearrange("p (bk e) -> p bk e", e=P),
                    in_=q_map.pop(pr)[:],
                )
                qT_map[pr] = t

            # plane r <-> k = 4*pi + r; u16 bit value and bf16 scale
            PLANES = [(2, 0.5), (1, 1.0), (512, 1.0 / 512), (256, 1.0 / 256)]
            NKT16 = KW // P    # 16 k-tiles of u16 words per block

            def extract(pr):
                # whole pair per op: cnq plane layout is (blk, kt16, e) so
                # input and output iterate identically (flat [128, 4096])
                cnq_map[pr] = [
                    pCN.tile([P, 2 * NKT16 * P], bf16, tag=f"cnq{r}",
                             name=f"cnq{r}_{pr}")
                    for r in range(4)
                ]
                planes = cnq_map[pr]
                qT = qT_map.pop(pr)
                for r, (mask, scale) in enumerate(PLANES):
                    if mask == 512:
                        nc.vector.tensor_scalar(
                            out=planes[r][:], in0=qT[:],
                            scalar1=512, scalar2=None, op0=GE,
                        )
                        continue
                    t = pT16.tile([P, 2 * KW], u16, tag="t16",
                                  name=f"t16_{pr}_{r}")
                    nc.vector.tensor_scalar(
                        out=t[:], in0=qT[:], scalar1=mask, scalar2=None,
                        op0=AND,
                    )
                    nc.vector.tensor_scalar(
                        out=planes[r][:], in0=t[:],
                        scalar1=scale, scalar2=None, op0=MUL,
                    )

            xcnT = pX.tile([P, 2 * EL], bf16, tag="xcnT", name="xcnT")
            # column = hh*EL + e  (channel-half on partitions)

            psB_map = {}

            def pair_mms(pr, psB):
                planes = cnq_map[pr]
                ps = [psB.tile([P, 2 * P], f32, tag=f"psb{h}",
                               name=f"psb{pr}_{h}") for h in range(2)]
                psB_map[pr] = ps
                for r in range(4):
                    cv = planes[r][:].rearrange(
                        "p (blk kt e) -> p blk kt e", blk=2, e=P)
                    for kt in range(NKT16):
                        for h in range(2):
                            nc.tensor.matmul(
                                ps[h][:],
                                h_view[:, h, r * NKT16 + kt, :],
                                cv[:, :, kt, :],
                                start=(r == 0 and kt == 0),
                                stop=(r == 3 and kt == NKT16 - 1),
                            )

            def xcn_copy(pr):
                ps = psB_map.pop(pr)
                del cnq_map[pr]
                for h in range(2):
                    nc.scalar.activation(
                        xcnT[0:P, h * EL + pr * 2 * P:h * EL + (pr + 1) * 2 * P],
                        ps[h][:], Ident,
                    )

            # ---- stage C ----
            prodT_map = {}
            xij_map = {}

            def gather_x(b):
                t = pCg.tile([P, 2 * C], bf16, tag="xij_g", name=f"xijg{b}")
                _rr(nc.gpsimd.indirect_dma_start(
                    out=t[:, 0:C], out_offset=None, in_=xnat_d[:, :],
                    in_offset=bass.IndirectOffsetOnAxis(
                        ap=ix[b][:, 0:1], axis=0),
                ))
                _rr(nc.gpsimd.indirect_dma_start(
                    out=t[:, C:2 * C], out_offset=None, in_=xnat_d[:, :],
                    in_offset=bass.IndirectOffsetOnAxis(
                        ap=ix[b][:, 1:2], axis=0),
                ))
                xij_map[b] = t

            def prod(b):
                t = pCg.tile([P, C], bf16, tag="prod", name=f"prod{b}")
                g = xij_map.pop(b)
                nc.vector.tensor_tensor(
                    out=t[:], in0=g[:, 0:C], in1=g[:, C:2 * C], op=MUL
                )
                return t

            def prodT(grp, tiles):
                W = CGRP * P
                t = pCt.tile([P, 2 * W], bf16, tag="prodT", name=f"prodT{grp}")
                tv = t[:].rearrange("p (blk hh e) -> p blk hh e", blk=CGRP, e=P)
                prodT_map[grp] = t
                for t2, pt in enumerate(tiles):
                    nc.sync.dma_start_transpose(out=tv[:, t2, :, :], in_=pt[:])

            def stage_c(grp, psC, psO):
                W = CGRP * P  # 512 edges
                r0 = xcnT[0:P, grp * W:(grp + 1) * W]
                r1 = xcnT[0:P, EL + grp * W:EL + (grp + 1) * W]
                pT = prodT_map.pop(grp)

                def mlp_layer(rhs2, wname, bname, outtag, packed,
                              tagas=None):
                    tagas = tagas or outtag
                    outs = []
                    for h in range(2):
                        ps = psC.tile([P, W], f32, tag="psc",
                                      name=f"psc_{grp}_{outtag}{h}")
                        if packed:
                            rhs_v = rhs2[:].rearrange(
                                "p (blk hh e) -> p blk hh e", blk=CGRP, e=P)
                            rr0, rr1 = rhs_v[:, :, 0, :], rhs_v[:, :, 1, :]
                        else:
                            rr0, rr1 = rhs2[0], rhs2[1]
                        nc.tensor.matmul(
                            ps[:], wC_sb[wname][0][:, h * P:(h + 1) * P],
                            rr0, start=True, stop=False,
                        )
                        nc.tensor.matmul(
                            ps[:], wC_sb[wname][1][:, h * P:(h + 1) * P],
                            rr1, start=False, stop=True,
                        )
                        t = pCm.tile([P, W], bf16, tag=f"{tagas}{h}",
                                    name=f"{outtag}{h}_{grp}")
                        nc.scalar.activation(
                            t[:], ps[:], Relu, bias=b_sb[bname][:, h:h + 1]
                        )
                        outs.append(t)
                    return outs

                def aps(ts):
                    return [t[:] for t in ts]

                xijT = mlp_layer(pT, "xij_w", "xij_b", "xijT", True)
                u1T = mlp_layer((r0, r1), "xcn_w1", "xcn_b1", "u1T", False)
                u2T = mlp_layer(aps(u1T), "xcn_w2", "xcn_b2", "u2T", False)
                zT = []
                for h in range(2):
                    zb = pCm.tile([P, W], bf16, tag=f"u1T{h}", name=f"zb{h}_{grp}")
                    nc.vector.tensor_tensor(
                        out=zb[:], in0=u2T[h][:],
                        in1=beta_sb.to_broadcast([P, W]), op=MUL,
                    )
                    zt = pCm.tile([P, W], bf16, tag=f"zT{h}", name=f"zT{h}_{grp}")
                    nc.vector.tensor_tensor(
                        out=zt[:], in0=zb[:], in1=xijT[h][:], op=ADD
                    )
                    zT.append(zt)
                vT = mlp_layer(aps(zT), "lin_w1", "lin_b1", "vT", False,
                               tagas="u2T")
                pso = psO.tile([1, W], f32, tag="pso", name=f"pso{grp}")
                nc.tensor.matmul(
                    pso[:], lw2_sb[0][:], vT[0][:], start=True, stop=False
                )
                nc.tensor.matmul(
                    pso[:], lw2_sb[1][:], vT[1][:], start=False, stop=True
                )
                nc.scalar.activation(
                    out_row[0:1, grp * W:(grp + 1) * W], pso[:],
                    Ident, bias=lb2_sb,
                )

            # ---- phased emission ----
            # Phase 1: stage A completely clean -- no B-chain op shares any
            # queue stage A depends on, so the A loop paces at the ACT rate.
            # Phase 2: B chain (gathers/AND/transpose/extract) pipelined with
            # the pair matmuls and stage C.
            with tc.tile_pool(name="psB", bufs=2, space="PSUM") as psB:
                with tc.tile_pool(name="stA", bufs=3) as pA, \
                     tc.tile_pool(name="stAy", bufs=4) as pAy, \
                     tc.tile_pool(name="psA", bufs=4, space="PSUM") as psA:
                    for _sg in range(8):
                        load_xt(_sg)
                    gather_x(0); gather_x(1); gather_x(2); gather_x(3)
                    gather_x(4); gather_x(5); gather_x(6); gather_x(7)
                    stage_a_group(0, pA, pAy, psA)
                    stage_a_group(1, pA, pAy, psA)
                    stage_a_group(2, pA, pAy, psA)
                    stage_a_group(3, pA, pAy, psA)
                    stage_a_group(4, pA, pAy, psA)
                    stage_a_group(5, pA, pAy, psA)
                    stage_a_group(6, pA, pAy, psA)
                    stage_a_group(7, pA, pAy, psA)
                    stage_a_group(8, pA, pAy, psA)
                    stage_a_group(9, pA, pAy, psA)
                    stage_a_group(10, pA, pAy, psA)
                    stage_a_group(11, pA, pAy, psA)
                    stage_a_group(12, pA, pAy, psA)
                    stage_a_group(13, pA, pAy, psA)
                    stage_a_group(14, pA, pAy, psA)
                    stage_a_group(15, pA, pAy, psA)
                    pt0 = [prod(b) for b in range(CGRP)]
                    prodT(0, pt0)
                    pt1 = [prod(b) for b in range(CGRP, 2 * CGRP)]
                    prodT(1, pt1)
                # Phase 2
                gather_adj(0); gather_adj(1)
                qprod(0)
                gather_adj(2)
                qtrans(0)
                qprod(1)
                extract(0)
                gather_adj(3)
                qtrans(1)
                qprod(2)
                extract(1)
                with tc.tile_pool(name="psC", bufs=2, space="PSUM") as psC, \
                     tc.tile_pool(name="psO", bufs=1, space="PSUM") as psO:
                    pair_mms(0, psB)
                    gather_adj(4)
                    qtrans(2)
                    qprod(3)
                    extract(2)
                    gather_adj(5)
                    qtrans(3)
                    qprod(4)
                    extract(3)
                    xcn_copy(0)
                    pair_mms(1, psB)
                    gather_adj(6)
                    qtrans(4)
                    qprod(5)
                    extract(4)
                    gather_adj(7)
                    qtrans(5)
                    qprod(6)
                    extract(5)
                    xcn_copy(1)
                    pair_mms(2, psB)
                    qtrans(6)
                    qprod(7)
                    extract(6)
                    qtrans(7)
                    extract(7)
                    xcn_copy(2)
                    stage_c(0, psC, psO)
                    pair_mms(3, psB)
                    xcn_copy(3)
                    stage_c(1, psC, psO)

            nc.sync.dma_start(out=out_d[:, :], in_=out_row[0:1, :])

    return _split_multi_waits(nc) if split_waits else nc


def kernel(**inputs):
    from concourse.bass_utils import run_bass_kernel_spmd

    if "nc" not in _CACHE:
        _CACHE["nc"] = _build()
    nc = _CACHE["nc"]

    x = np.ascontiguousarray(inputs["x"], dtype=np.float32)
    adj = np.asarray(inputs["adj"], dtype=np.float32)
    tar = np.asarray(inputs["tar_ei"]).astype(np.int64)

    # byte kp = 2*A[:, 2kp] + A[:, 2kp+1]; u16 word = byte pair (LE), so
    # word pi encodes k = 4pi..4pi+3 at bit values (2, 1, 512, 256)
    ab = (2.0 * adj[:, 0::2] + adj[:, 1::2]).astype(np.uint8)
    adj2 = np.ascontiguousarray(ab).view(np.uint16)
    # node permutation: k = 4pi + r lives in quarter r, row pi
    xp = np.concatenate([x[0::4], x[1::4], x[2::4], x[3::4]], axis=0)

    def btile(b):
        return np.ascontiguousarray(
            np.asarray(b, dtype=np.float32).reshape(2, P).T)

    def wtile(w):
        # [256, M] -> [128, 2*M] with the two k-halves side by side
        w = np.asarray(w, dtype=np.float32)
        return w.reshape(2, P, -1).transpose(1, 0, 2).reshape(P, -1)

    wall = np.concatenate(
        [wtile(inputs[n]) for n in
         ("xlin_w1", "xlin_w2", "xcn_w1", "xcn_w2", "xij_w", "lin_w1")]
        + [wtile(inputs["lin_w2"])], axis=1)
    ball = np.concatenate(
        [btile(inputs[n]) for n in
         ("xlin_b1", "xlin_b2", "xcn_b1", "xcn_b2", "xij_b", "lin_b1")]
        + [np.full((P, 1), np.asarray(inputs["beta"]).reshape(-1)[0],
                   dtype=np.float32),
           np.full((P, 1), np.asarray(inputs["lin_b2"]).reshape(-1)[0],
                   dtype=np.float32)], axis=1)

    common = {
        "xnat": np.ascontiguousarray(xp).astype(ml_dtypes.bfloat16),
        "xT": np.ascontiguousarray(xp.T).astype(ml_dtypes.bfloat16),
        "adj2": np.ascontiguousarray(adj2),
        "wall": np.ascontiguousarray(wall).astype(ml_dtypes.bfloat16),
        "ball": np.ascontiguousarray(ball),
    }

    in_maps = []
    for c in range(NCORES):
        t = tar[:, c * EL:(c + 1) * EL]              # [2, 1024]
        # adj row indices (original node ids), interleaved (i,j) per block
        ta = t.reshape(2, NB, P)                     # [t, b, p]
        idx_adj = np.transpose(ta, (2, 1, 0)).reshape(P, 2 * NB)
        # xnat row indices (permuted positions)
        tp = (t >> 2) + (N // 4) * (t & 3)
        tx = tp.reshape(2, NB, P)
        idx_x = np.transpose(tx, (2, 1, 0)).reshape(P, 2 * NB)
        m = dict(common)
        m["idx"] = np.ascontiguousarray(
            np.concatenate([idx_adj, idx_x], axis=1).astype(np.int32))
        in_maps.append(m)

    res = run_bass_kernel_spmd(
        nc, in_maps, core_ids=list(range(NCORES)), trace=TRACE
    )
    global LAST_RESULT
    LAST_RESULT = res
    out = np.concatenate(
        [res.results[c]["out"].reshape(EL, 1) for c in range(NCORES)], axis=0
    )
    return out.astype(np.float32)
